# revision 2
# baseline (speedup 1.0000x reference)
"""MoE feed-forward (8 experts, top-2, D=1024, H=4096) on 8 Trainium2 cores.

Strategy: expert-parallel with host-side routing, fp16 matmuls.
  - Host computes the gating (logits -> top-2 -> softmax) in fp64 and
    gathers each expert's tokens into a padded activation matrix laid
    out for the device: xh [128, 8, C] fp16 (partition = d%128).
  - Core e runs the dense FFN for expert e over its routed tokens:
        y = gelu(x @ w1[e] + b1[e]) @ w2[e]
    Both weight matrices live fully resident in SBUF in fp16 (16 MB),
    streamed in chunks at the start of each rep so the first token
    tile's compute overlaps the tail of the weight DMA. For each
    384-token tile the full hidden dim (32 j-blocks) is computed and
    gelu'd into an fp16 hT tile, then the second matmul accumulates
    all 32 j-blocks in PSUM, so y is written once (no partial sums).
  - Host combines: out[tok] += p_e * (y + b2[e]).

Self-contained: hardcodes all shapes from the problem spec.
"""

import numpy as np

import concourse.bass as bass
import concourse.mybir as mybir
import concourse.tile as tile
from concourse.bass_utils import run_bass_kernel_spmd

F32 = mybir.dt.float32
F16 = mybir.dt.float16

D_MODEL = 1024
HIDDEN = 4096
N_EXPERTS = 8
TOP_K = 2
DBLK = D_MODEL // 128     # 8
JBLK = HIDDEN // 128      # 32
CT = 384                  # token tile (matmul1 moving free dim, 3*128)


# ---------------------------------------------------------------------------
# Walrus workaround: this container's Tile emits instructions carrying more
# sync waits than the bundled walrus accepts ("Too many sync wait commands").
# Hoist excess waits onto EventSemaphore instructions placed immediately
# before the overloaded instruction (same engine, same block) — semantically
# identical: the engine blocks on each wait in program order.
_CAP_BY_OPCODE = {"EventSemaphore": 2}
_DEFAULT_CAP = 1
_split_counter = [0]


def split_excess_waits(nc):
    for f in nc.m.functions:
        for bb in f.blocks:
            new_insts = []
            changed = False
            for inst in bb.instructions:
                si = inst.sync_info
                waits = list(si.on_wait) if si is not None else []
                cap = _CAP_BY_OPCODE.get(inst.opcode, _DEFAULT_CAP)
                if len(waits) > cap:
                    changed = True
                    excess, keep = waits[:-cap], waits[-cap:]
                    for i in range(0, len(excess), 2):
                        _split_counter[0] += 1
                        new_insts.append(mybir.InstEventSemaphore(
                            name=f"I-waitsplit-{_split_counter[0]}",
                            engine=inst.engine,
                            sync_info=mybir.SyncInfo(
                                on_wait=excess[i:i + 2], on_update=[]),
                        ))
                    inst.sync_info = mybir.SyncInfo(
                        on_wait=keep, on_update=list(si.on_update))
                new_insts.append(inst)
            if changed:
                bb.instructions = new_insts
    return nc


# ---------------------------------------------------------------------------
def build_nc(C, act=None, reps=1):
    """Per-core FFN program: xh [128, 8, C] fp16 -> y [C, D] f32."""
    if act is None:
        act = mybir.ActivationFunctionType.Gelu
    assert C % CT == 0
    NCT = C // CT
    nc = bass.Bass()
    xh = nc.dram_tensor("xh", [128, DBLK, C], F16, kind="ExternalInput")
    # w1h[p, c, d, hh] = w1[d*128+p, c*512+hh] — 8 h-chunks of 512
    w1h = nc.dram_tensor("w1h", [128, 8, DBLK, 512], F16, kind="ExternalInput")
    # w2h[p, j, dd] = w2[j*128+p, dd]
    w2h = nc.dram_tensor("w2h", [128, JBLK, D_MODEL], F16, kind="ExternalInput")
    b1 = nc.dram_tensor("b1", [HIDDEN], F32, kind="ExternalInput")
    y = nc.dram_tensor("y", [C, D_MODEL], F32, kind="ExternalOutput")

    with tile.TileContext(nc) as tc:
        with (
            tc.tile_pool(name="wpool", bufs=1) as wpool,
            tc.tile_pool(name="xpool", bufs=3) as xpool,
            tc.tile_pool(name="hpool", bufs=1) as hpool,
            tc.tile_pool(name="ypool", bufs=4) as ypool,
            tc.tile_pool(name="ps1", bufs=3, space="PSUM") as ps1,
            tc.tile_pool(name="ps2", bufs=2, space="PSUM") as ps2,
        ):
            def whole(_=None):
                b1t = wpool.tile([128, JBLK], F32, tag="b1t")
                nc.sync.dma_start(
                    out=b1t[:],
                    in_=b1.ap().rearrange("(b p) -> p b", p=128))
                # prefetch first two token tiles ahead of the weight bulk
                xts = {}
                for t in range(min(2, NCT)):
                    xt = xpool.tile([128, DBLK, CT], F16, tag="xt")
                    nc.sync.dma_start(
                        out=xt[:], in_=xh.ap()[:, :, t * CT:(t + 1) * CT])
                    xts[t] = xt
                # weights fully resident; chunked so tile-0 compute overlaps
                w1t = wpool.tile([128, 8, DBLK, 512], F16, tag="w1t")
                for cchunk in range(8):
                    nc.sync.dma_start(
                        out=w1t[:, cchunk], in_=w1h.ap()[:, cchunk])
                w2t = wpool.tile([128, JBLK, D_MODEL], F16, tag="w2t")
                for q in range(4):
                    nc.sync.dma_start(
                        out=w2t[:, 8 * q:8 * q + 8],
                        in_=w2h.ap()[:, 8 * q:8 * q + 8])

                for t in range(NCT):
                    if t in xts:
                        xt = xts.pop(t)
                    else:
                        xt = xpool.tile([128, DBLK, CT], F16, tag="xt")
                        nc.sync.dma_start(
                            out=xt[:], in_=xh.ap()[:, :, t * CT:(t + 1) * CT])

                    # matmul1 + gelu over the full hidden dim
                    hT = hpool.tile([128, JBLK, CT], F16, tag="hT")
                    for j in range(JBLK):
                        cchunk, jj = divmod(j, 4)
                        ps = ps1.tile([128, CT], F32, tag="ps")
                        for d in range(DBLK):
                            nc.tensor.matmul(
                                ps[:],
                                w1t[:, cchunk, d, jj * 128:(jj + 1) * 128],
                                xt[:, d, :],
                                start=(d == 0), stop=(d == DBLK - 1))
                        nc.scalar.activation(
                            hT[:, j, :], ps[:], act, bias=b1t[:, j:j + 1])

                    # matmul2: accumulate all 32 j-blocks in PSUM, store once
                    for cs in range(CT // 128):
                        p2a = ps2.tile([128, 512], F32, tag="p2a")
                        for j in range(JBLK):
                            nc.tensor.matmul(
                                p2a[:],
                                hT[:, j, cs * 128:(cs + 1) * 128],
                                w2t[:, j, 0:512],
                                start=(j == 0), stop=(j == JBLK - 1))
                        p2b = ps2.tile([128, 512], F32, tag="p2b")
                        for j in range(JBLK):
                            nc.tensor.matmul(
                                p2b[:],
                                hT[:, j, cs * 128:(cs + 1) * 128],
                                w2t[:, j, 512:1024],
                                start=(j == 0), stop=(j == JBLK - 1))
                        yb = ypool.tile([128, D_MODEL], F32, tag="yb")
                        nc.vector.tensor_copy(yb[:, 0:512], p2a[:])
                        nc.vector.tensor_copy(yb[:, 512:1024], p2b[:])
                        nc.sync.dma_start(
                            out=y.ap()[t * CT + cs * 128:
                                       t * CT + (cs + 1) * 128, :],
                            in_=yb[:])

            if reps == 1:
                whole()
            else:
                with tc.For_i(0, reps, 1):
                    whole()
    return nc


# ---------------------------------------------------------------------------
def _gating(x2d, gate_w, gate_b):
    """fp64 host gating; returns per-expert (idx, prob) matching jax top_k
    (ties -> lower index, measure-zero for random inputs)."""
    logits = x2d.astype(np.float64) @ gate_w.astype(np.float64) \
        + gate_b.astype(np.float64)
    i1 = np.argmax(logits, axis=-1)
    n = len(logits)
    ar = np.arange(n)
    v1 = logits[ar, i1]
    l2 = logits.copy()
    l2[ar, i1] = -np.inf
    i2 = np.argmax(l2, axis=-1)
    v2 = l2[ar, i2]
    m = np.maximum(v1, v2)
    e1 = np.exp(v1 - m)
    e2 = np.exp(v2 - m)
    s = e1 + e2
    p1 = (e1 / s)
    p2 = (e2 / s)
    out = []
    for e in range(N_EXPERTS):
        m1 = i1 == e
        m2 = i2 == e
        idx = np.nonzero(m1 | m2)[0]
        prob = np.where(m1, p1, p2)[idx].astype(np.float32)
        out.append((idx, prob))
    return out


def make_in_maps(x2d, routes, w1, b1, w2, C):
    """Build the per-core device input dict list."""
    in_maps = []
    for e in range(N_EXPERTS):
        idx, _ = routes[e]
        xpad = np.zeros((C, D_MODEL), dtype=np.float32)
        xpad[:len(idx)] = x2d[idx]
        # xh[p, b, c] = x[c, b*128+p]
        xhe = np.ascontiguousarray(
            xpad.T.reshape(DBLK, 128, C).transpose(1, 0, 2)).astype(np.float16)
        w1e = np.ascontiguousarray(
            w1[e].reshape(DBLK, 128, 8, 512).transpose(1, 2, 0, 3)
        ).astype(np.float16)
        w2e = np.ascontiguousarray(
            w2[e].reshape(JBLK, 128, D_MODEL).transpose(1, 0, 2)
        ).astype(np.float16)
        in_maps.append({
            "xh": xhe,
            "w1h": w1e,
            "w2h": w2e,
            "b1": np.ascontiguousarray(b1[e], dtype=np.float32),
        })
    return in_maps


_NC_CACHE = {}


def kernel(x, gate_w, gate_b, w1, b1, w2, b2):
    x = np.asarray(x, dtype=np.float32)
    gate_w = np.asarray(gate_w, dtype=np.float32)
    gate_b = np.asarray(gate_b, dtype=np.float32)
    w1 = np.asarray(w1, dtype=np.float32)
    b1 = np.asarray(b1, dtype=np.float32)
    w2 = np.asarray(w2, dtype=np.float32)
    b2 = np.asarray(b2, dtype=np.float32)

    B, T, D = x.shape
    x2d = x.reshape(-1, D)
    routes = _gating(x2d, gate_w, gate_b)

    max_n = max(len(idx) for idx, _ in routes)
    C = max(6 * CT, -(-max_n // CT) * CT)

    if C not in _NC_CACHE:
        nc = build_nc(C)
        split_excess_waits(nc)
        _NC_CACHE[C] = nc
    nc = _NC_CACHE[C]

    in_maps = make_in_maps(x2d, routes, w1, b1, w2, C)
    res = run_bass_kernel_spmd(nc, in_maps, core_ids=list(range(N_EXPERTS)))

    out2d = np.zeros((B * T, D_MODEL), dtype=np.float32)
    for e in range(N_EXPERTS):
        idx, prob = routes[e]
        n = len(idx)
        y_e = res.results[e]["y"][:n] + b2[e]
        out2d[idx] += prob[:, None] * y_e
    return out2d.reshape(B, T, D_MODEL)
